# revision 1
# baseline (speedup 1.0000x reference)
"""Multi-head self-attention with positional bias, sharded over 8 NeuronCores.

Sharding: head-parallel. Core h computes head h for all batches:
  q/k/v projections with the head's weight slices, scores + softmax with the
  head's pos_bias slice, and the partial output  o_h @ Wout[h*64:(h+1)*64, :].
The full output is the sum of the 8 partials (row-parallel Wout).

Device kernel math (per core):
  - query is supplied pre-transposed (qT [D, B*N]) so the contraction dim of
    every projection lands on SBUF partitions.
  - scores are computed TRANSPOSED: ST[j, i] = bias[i, j] + k_j . q_i, so exp's
    output P~[j, i] is directly the layout the attention*V matmul needs (no P
    transposes). The bias lands in PSUM via an identity matmul (start=True),
    the qk matmul accumulates on top.
  - exp skips max-subtraction: scores are ~N(0, 2) (bounded), exp is safe in
    fp32 and softmax is shift-invariant.
  - softmax denominator: ones column appended to v (safe mode) or packed
    ones-matmuls (packed mode); normalization is deferred to the PSUM
    evacuation after the Wout matmul (per-partition scalar multiply).
  - all matmuls run in float32r (fp22 mantissa truncation, full PE speed at
    free-dim >= 256, fp32 accumulation): rel err ~1e-4.
"""

import numpy as np
from contextlib import ExitStack

import concourse.bass as bass
import concourse.bacc as bacc
import concourse.mybir as mybir
import concourse.tile as tile
from concourse.bass_utils import run_bass_kernel_spmd
from concourse.masks import make_identity

HEADS = 8
DH = 64
B, N, D = 4, 2048, 512
SCALE = DH ** -0.5
N_CORES = 8
PACKED = False  # shared-PSUM-bank packing tricks (col-strip oT, 4-way denom)

F32 = mybir.dt.float32
F32R = mybir.dt.float32r


def build_nc(b=B, n=N, d=D, packed=PACKED, n_cores=1):
    """Build the per-core Bass program. All cores run the same program (SPMD);
    per-head differences come in through the input tensors."""
    assert b % 2 == 0 and n % 512 == 0 and d % 128 == 0
    T = b * n           # total tokens
    CC = d // 128       # contraction chunks for the projections
    NJ = n // 128       # key tiles (j)
    NIC = n // 512      # query chunks of 512 (i)
    assert NIC % 2 == 0
    NIP = NIC // 2      # i-groups of 1024 (one exp op each)
    NPAIR = b // 2
    IC = 512
    VW = 64 if packed else 65  # v block width (safe mode: +1 ones column)

    nc = bacc.Bacc("TRN2", target_bir_lowering=False, debug=False,
                   num_devices=n_cores)
    qT = nc.declare_dram_parameter("qT", [d, T], F32R, isOutput=False)
    biasT = nc.declare_dram_parameter("biasT", [n, n], F32R, isOutput=False)
    wq = nc.declare_dram_parameter("wq", [d, DH], F32R, isOutput=False)
    wk = nc.declare_dram_parameter("wk", [d, DH], F32R, isOutput=False)
    wv = nc.declare_dram_parameter("wv", [d, DH], F32R, isOutput=False)
    wout = nc.declare_dram_parameter("wout", [DH, d], F32R, isOutput=False)
    out = nc.declare_dram_parameter("out", [T, d], F32, isOutput=True)

    with ExitStack() as ctx:
        tc = ctx.enter_context(tile.TileContext(nc))

        const = ctx.enter_context(tc.tile_pool(name="const", bufs=1))
        qk_pool = ctx.enter_context(tc.tile_pool(name="qkT", bufs=1))
        v_pool = ctx.enter_context(tc.tile_pool(name="v", bufs=1))
        ot_sb_pool = ctx.enter_context(tc.tile_pool(name="ot_sb", bufs=1))
        p_pool = ctx.enter_context(tc.tile_pool(name="pexp", bufs=4))
        out_pool = ctx.enter_context(tc.tile_pool(name="osb", bufs=6))

        ident_f32 = const.tile([128, 128], F32, tag="ident_f32")
        make_identity(nc, ident_f32)
        ident = const.tile([128, 128], F32R, tag="ident")
        nc.vector.tensor_copy(ident, ident_f32)
        zbias = const.tile([128, 1], F32, tag="zbias")
        nc.vector.memset(zbias, 0.0)
        ones16 = const.tile([128, 16], F32, tag="ones16")
        nc.vector.memset(ones16, 1.0)
        if packed:
            ones32 = const.tile([128, 32], F32R, tag="ones32")
            nc.vector.tensor_copy(ones32[:, 0:16], ones16)
            nc.vector.tensor_copy(ones32[:, 16:32], ones16)

        w_sb = {}
        for name, w in (("wq", wq), ("wk", wk), ("wv", wv)):
            t = const.tile([128, CC, DH], F32R, tag=name)
            nc.sync.dma_start(out=t, in_=w[:, :].rearrange("(c p) e -> p c e", p=128))
            w_sb[name] = t
        wout_sb = const.tile([128, d], F32R, tag="wout")
        nc.sync.dma_start(out=wout_sb[0:64, :], in_=wout[:, :])
        nc.sync.dma_start(out=wout_sb[64:128, :], in_=wout[:, :])

        qT_sb = [qk_pool.tile([128, n], F32R, tag=f"qT{p}", name=f"qT{p}") for p in range(NPAIR)]
        kT_sb = [qk_pool.tile([128, n], F32R, tag=f"kT{p}", name=f"kT{p}") for p in range(NPAIR)]
        v_sb = [v_pool.tile([128, NJ * VW], F32R, tag=f"v{bb}", name=f"v{bb}") for bb in range(b)]
        if not packed:
            for bb in range(b):
                ones_cols = v_sb[bb].rearrange("p (t w) -> p t w", w=VW)[:, :, DH:VW]
                nc.vector.tensor_copy(ones_cols, ones16[:, 0:NJ].rearrange("p (t o) -> p t o", o=1))
        ot_sb = [ot_sb_pool.tile([128, n], F32R, tag=f"ot{p}", name=f"ot{p}") for p in range(NPAIR)]

        # denominator staging: row bb lives at partition 32*bb (engines need
        # 32-aligned partition bases)
        den_all = const.tile([32 * (b - 1) + 1, n], F32, tag="den_all")
        den_sb = [den_all[32 * bb:32 * bb + 1, :] for bb in range(b)]
        recip_in = [const.tile([128, NJ], F32, tag=f"recip_in{bb}", name=f"ri{bb}")
                    for bb in range(b)]
        recip_sb = [const.tile([128, NJ], F32, tag=f"recip_sb{bb}", name=f"rs{bb}")
                    for bb in range(b)]

        # ---------------- projections (per batch) ----------------
        HN = max(n // 4, 512)  # qt chunk width (>= one projection rhs slice)
        NQ = n // HN
        with tc.tile_pool(name="qt", bufs=3 * CC) as qt_pool, \
             tc.tile_pool(name="pqk", bufs=4, space="PSUM") as pqk_pool, \
             tc.tile_pool(name="pv", bufs=4, space="PSUM") as pv_pool:
            for bb in range(b):
                pair, lb = bb // 2, bb % 2
                rows = slice(64 * lb, 64 * lb + 64)
                for hh in range(NQ):
                    qt_c = []
                    for c in range(CC):
                        t = qt_pool.tile([128, HN], F32R, tag="qt", name="qtc")
                        nc.sync.dma_start(
                            out=t, in_=qT[c * 128:(c + 1) * 128,
                                          bb * n + hh * HN: bb * n + (hh + 1) * HN])
                        qt_c.append(t)
                    for wname, dest in (("wq", qT_sb[pair]), ("wk", kT_sb[pair])):
                        for hic in range(HN // IC):
                            icc = (hh * HN + hic * IC) // IC
                            ps = pqk_pool.tile([64, IC], F32, tag="pqk")
                            for c in range(CC):
                                nc.tensor.matmul(
                                    ps, lhsT=w_sb[wname][:, c, :],
                                    rhs=qt_c[c][:, hic * IC:(hic + 1) * IC],
                                    start=(c == 0), stop=(c == CC - 1))
                            nc.vector.tensor_copy(dest[rows, icc * IC:(icc + 1) * IC], ps)
                    for htt in range(HN // 128):
                        tt = (hh * HN + htt * 128) // 128
                        psv = pv_pool.tile([128, DH], F32, tag="pv")
                        for c in range(CC):
                            nc.tensor.matmul(
                                psv, lhsT=qt_c[c][:, htt * 128:(htt + 1) * 128],
                                rhs=w_sb["wv"][:, c, :],
                                start=(c == 0), stop=(c == CC - 1))
                        nc.vector.tensor_copy(v_sb[bb][:, tt * VW: tt * VW + DH], psv)

        # ---------------- scores + softmax + P~^T V ----------------
        with tc.tile_pool(name="bias", bufs=NJ) as bias_pool, \
             tc.tile_pool(name="st", bufs=2, space="PSUM") as st_pool, \
             tc.tile_pool(name="ot", bufs=2 if packed else 4, space="PSUM") as ot_pool, \
             tc.tile_pool(name="dn", bufs=1, space="PSUM") as dn_pool:
            for ip in range(NIP):
                bias_t = []
                for jt in range(NJ):
                    t = bias_pool.tile([128, 2 * IC], F32R, tag="bias")
                    nc.sync.dma_start(
                        out=t, in_=biasT[jt * 128:(jt + 1) * 128, ip * 2 * IC:(ip + 1) * 2 * IC])
                    bias_t.append(t)
                for pair in range(NPAIR):
                    if packed:
                        ot_ps = [ot_pool.tile([128, IC], F32, tag="ot", name="otp")
                                 for _ in range(2)]
                        dn_ps = dn_pool.tile([128, IC], F32, tag="dn")
                    else:
                        ot_ps = {(lb, il): ot_pool.tile([65, IC], F32, tag="ot", name="otp")
                                 for lb in range(2) for il in range(2)}
                    for jt in range(NJ):
                        for lb in range(2):
                            bb = 2 * pair + lb
                            rows = slice(64 * lb, 64 * lb + 64)
                            st = st_pool.tile([128, 2 * IC], F32, tag="st")
                            for il in range(2):
                                cols = slice(il * IC, (il + 1) * IC)
                                ic = ip * 2 + il
                                nc.tensor.matmul(
                                    st[:, cols], lhsT=ident, rhs=bias_t[jt][:, cols],
                                    start=True, stop=False)
                                nc.tensor.matmul(
                                    st[:, cols],
                                    lhsT=kT_sb[pair][rows, jt * 128:(jt + 1) * 128],
                                    rhs=qT_sb[pair][rows, ic * IC:(ic + 1) * IC],
                                    start=False, stop=True)
                            pexp = p_pool.tile([128, 2 * IC], F32R, tag="pexp")
                            nc.scalar.activation(
                                pexp, st, mybir.ActivationFunctionType.Exp, bias=zbias)
                            for il in range(2):
                                pcols = slice(il * IC, (il + 1) * IC)
                                if packed:
                                    nc.tensor.matmul(
                                        ot_ps[il][rows, :],
                                        lhsT=v_sb[bb][:, jt * VW: jt * VW + DH],
                                        rhs=pexp[:, pcols],
                                        start=(jt == 0 and lb == 0),
                                        stop=(jt == NJ - 1 and lb == 1),
                                        skip_group_check=True)
                                    s_idx = il * 2 + lb
                                    nc.tensor.matmul(
                                        dn_ps[32 * s_idx: 32 * s_idx + 32, :],
                                        lhsT=ones32, rhs=pexp[:, pcols],
                                        start=(jt == 0 and s_idx == 0),
                                        stop=(jt == NJ - 1 and s_idx == 3),
                                        tile_position=(0, 32 * s_idx),
                                        skip_group_check=True)
                                else:
                                    nc.tensor.matmul(
                                        ot_ps[(lb, il)],
                                        lhsT=v_sb[bb][:, jt * VW: jt * VW + VW],
                                        rhs=pexp[:, pcols],
                                        start=(jt == 0), stop=(jt == NJ - 1))
                    # evacuate oT + denominators for this (ip, pair)
                    for il in range(2):
                        ic = ip * 2 + il
                        ccols = slice(ic * IC, (ic + 1) * IC)
                        if packed:
                            for lb in range(2):
                                s_idx = il * 2 + lb
                                bb = 2 * pair + lb
                                nc.vector.tensor_copy(
                                    den_sb[bb][0:1, ccols],
                                    dn_ps[32 * s_idx: 32 * s_idx + 1, :])
                            nc.vector.tensor_copy(ot_sb[pair][:, ccols], ot_ps[il])
                        else:
                            for lb in range(2):
                                bb = 2 * pair + lb
                                rows = slice(64 * lb, 64 * lb + 64)
                                nc.vector.tensor_copy(
                                    den_sb[bb][0:1, ccols], ot_ps[(lb, il)][64:65, :])
                                nc.vector.tensor_copy(
                                    ot_sb[pair][rows, ccols], ot_ps[(lb, il)][0:64, :])

        # denominator rows -> per-token-tile columns (via DRAM bounce), reciprocal
        for bb in range(b):
            den_dram = nc.dram_tensor(f"den_dram{bb}", [n], F32)
            nc.sync.dma_start(out=den_dram[:], in_=den_sb[bb][0:1, :])
            nc.sync.dma_start(
                out=recip_in[bb],
                in_=den_dram[:].rearrange("(t p) -> p t", p=128))
            nc.vector.reciprocal(recip_sb[bb], recip_in[bb])

        # ---------------- output projection ----------------
        with tc.tile_pool(name="po", bufs=6, space="PSUM") as po_pool:
            for pair in range(NPAIR):
                for tg in range(NJ):
                    for lb in range(2):
                        bb = 2 * pair + lb
                        rows = slice(64 * lb, 64 * lb + 64)
                        po = po_pool.tile([128, d], F32, tag="po")
                        nc.tensor.matmul(
                            po, lhsT=ot_sb[pair][rows, tg * 128:(tg + 1) * 128],
                            rhs=wout_sb[rows, :], start=True, stop=True)
                        osb = out_pool.tile([128, d], F32, tag="osb")
                        nc.vector.tensor_scalar_mul(
                            osb, po, recip_sb[bb][:, tg: tg + 1])
                        nc.sync.dma_start(
                            out=out[bb * n + tg * 128: bb * n + (tg + 1) * 128, :],
                            in_=osb)
    nc.compile()
    return nc


def make_in_maps(query, pos_bias, Wq, Wk, Wv, Wout, n_cores=N_CORES):
    """Host-side sharding/layout prep. Head h -> core h."""
    query = np.asarray(query, dtype=np.float32)
    pos_bias = np.asarray(pos_bias, dtype=np.float32)
    Wq = np.asarray(Wq, dtype=np.float32)
    Wk = np.asarray(Wk, dtype=np.float32)
    Wv = np.asarray(Wv, dtype=np.float32)
    Wout = np.asarray(Wout, dtype=np.float32)

    b, n, d = query.shape
    qT = np.ascontiguousarray(query.reshape(b * n, d).T)
    wq_s = Wq * np.float32(SCALE)
    in_maps = []
    for h in range(n_cores):
        sl = slice(h * DH, (h + 1) * DH)
        in_maps.append({
            "qT": qT,
            "biasT": np.ascontiguousarray(pos_bias[h].T),
            "wq": np.ascontiguousarray(wq_s[:, sl]),
            "wk": np.ascontiguousarray(Wk[:, sl]),
            "wv": np.ascontiguousarray(Wv[:, sl]),
            "wout": np.ascontiguousarray(Wout[sl, :]),
        })
    return in_maps


def run_device(in_maps, b=B, n=N, d=D, packed=PACKED, trace=False, **kw):
    nc = build_nc(b, n, d, packed, n_cores=len(in_maps))
    return run_bass_kernel_spmd(nc, in_maps, list(range(len(in_maps))), trace=trace, **kw)


def assemble(results, b=B, n=N, d=D):
    acc = np.zeros((b * n, d), dtype=np.float32)
    for r in results:
        acc += r["out"]
    return acc.reshape(b, n, d)


def kernel(query, pos_bias, Wq, Wk, Wv, Wout):
    in_maps = make_in_maps(query, pos_bias, Wq, Wk, Wv, Wout)
    res = run_device(in_maps)
    return assemble(res.results)



# revision 6
# speedup vs baseline: 1.1933x; 1.1933x over previous
"""Multi-head self-attention with positional bias, sharded over 8 NeuronCores.

Sharding: head-parallel. Core h computes head h for all batches; the full
output is the sum of the 8 per-core partials (row-parallel Wout), summed on
host in fp32.

v2 design (driven by the TimelineSim cost model, where a matmul costs
out_free_size * pe_cycle and engine element ops cost free_size * cycle_t):
  - everything bf16 on the wires (qT, bias, weights, pexp, oT, out); fp32
    only in PSUM accumulation and the exp input.
  - Wq/Wk merged into one [d, 128] projection matmul (halves proj MM count).
  - scores computed transposed ST[j, i] = k_j . q_i + bias[i, j]; the bias
    lands via EITHER an identity matmul on PE (start=True) OR a DVE
    scalar_tensor_tensor add staged through SBUF -- split by ALPHA to balance
    the PE and DVE engines.
  - exp on ACT (the hard floor: ~133us for 16.8M elements), 1024-wide ops.
  - softmax denominator: ones column 64 in v (costs nothing extra on PE);
    the oT evacuation keeps the den row in the same bf16 tile; a SBUF->SBUF
    transpose DMA turns den rows into per-token-tile columns for reciprocal.
  - loop order (ip, pair, lb) so only 2 oT accumulators are live -> PSUM fits
    st double-buffering (4) + ot (2) + out-proj po (2) = 8 banks.
  - out-projection + normalization + store pipelined per (ip, pair, lb).
"""

import numpy as np
import ml_dtypes
from contextlib import ExitStack

import concourse.bass as bass
import concourse.bacc as bacc
import concourse.mybir as mybir
import concourse.tile as tile
from concourse.bass_utils import run_bass_kernel_spmd
from concourse.masks import make_identity

HEADS = 8
DH = 64
B, N, D = 4, 2048, 512
SCALE = DH ** -0.5
N_CORES = 8

# fraction of (jt, lb) score tiles whose bias-add runs as a PE identity
# matmul; the rest run as DVE adds staged through SBUF.
ALPHA = 0.5

F32 = mybir.dt.float32
BF16 = mybir.dt.bfloat16
BF16NP = ml_dtypes.bfloat16


def build_nc(b=B, n=N, d=D, alpha=None, n_cores=1):
    """Build the per-core Bass program (SPMD; per-head data via inputs)."""
    if alpha is None:
        alpha = ALPHA
    assert b % 2 == 0 and n % 1024 == 0 and d % 128 == 0
    T = b * n
    CC = d // 128        # contraction chunks for projections
    NJ = n // 128        # key tiles (j)
    NIP = n // 1024      # i-windows of 1024
    NPAIR = b // 2
    VW = 65              # v block width (ones column at 64)

    nc = bacc.Bacc("TRN2", target_bir_lowering=False, debug=False,
                   num_devices=n_cores)
    qT = nc.declare_dram_parameter("qT", [d, T], BF16, isOutput=False)
    biasT = nc.declare_dram_parameter("biasT", [n, n], BF16, isOutput=False)
    wqk = nc.declare_dram_parameter("wqk", [d, 128], BF16, isOutput=False)
    wv = nc.declare_dram_parameter("wv", [d, DH], BF16, isOutput=False)
    wout = nc.declare_dram_parameter("wout", [DH, d], BF16, isOutput=False)
    out = nc.declare_dram_parameter("out", [T, d], BF16, isOutput=True)

    with ExitStack() as ctx:
        tc = ctx.enter_context(tile.TileContext(nc))

        const = ctx.enter_context(tc.tile_pool(name="const", bufs=1))
        qk_pool = ctx.enter_context(tc.tile_pool(name="qkT", bufs=1))
        v_pool = ctx.enter_context(tc.tile_pool(name="v", bufs=1))
        ot_sb_pool = ctx.enter_context(tc.tile_pool(name="ot_sb", bufs=1))
        bias_pool = ctx.enter_context(tc.tile_pool(name="bias", bufs=1))
        s_pool = ctx.enter_context(tc.tile_pool(name="s_sb", bufs=3))
        p_pool = ctx.enter_context(tc.tile_pool(name="pexp", bufs=4))
        out_pool = ctx.enter_context(tc.tile_pool(name="osb", bufs=6))

        ident_f32 = const.tile([128, 128], F32, tag="ident_f32")
        make_identity(nc, ident_f32)
        ident = const.tile([128, 128], BF16, tag="ident")
        nc.vector.tensor_copy(ident, ident_f32)
        zbias = const.tile([128, 1], F32, tag="zbias")
        nc.vector.memset(zbias, 0.0)

        wqk_sb = const.tile([128, CC, 128], BF16, tag="wqk")
        nc.sync.dma_start(out=wqk_sb, in_=wqk[:, :].rearrange("(c p) e -> p c e", p=128))
        wv_sb = const.tile([128, CC, DH], BF16, tag="wv")
        nc.sync.dma_start(out=wv_sb, in_=wv[:, :].rearrange("(c p) e -> p c e", p=128))
        wout_sb = const.tile([64, d], BF16, tag="wout")
        nc.sync.dma_start(out=wout_sb, in_=wout[:, :])

        qT_sb = [qk_pool.tile([128, n], BF16, tag=f"qT{p}", name=f"qT{p}") for p in range(NPAIR)]
        kT_sb = [qk_pool.tile([128, n], BF16, tag=f"kT{p}", name=f"kT{p}") for p in range(NPAIR)]
        v_sb = [v_pool.tile([128, NJ, VW], BF16, tag=f"v{bb}", name=f"v{bb}") for bb in range(b)]
        for bb in range(b):
            nc.vector.memset(v_sb[bb][:, :, DH:VW], 1.0)
        # oT + den row, per (pair, lb): rows 0..63 = oT (dh), row 64 = denom
        ot65 = [[ot_sb_pool.tile([VW, n], BF16, tag=f"ot{p}{l}", name=f"ot{p}{l}")
                 for l in range(2)] for p in range(NPAIR)]
        den_in = [const.tile([128, NJ], BF16, tag=f"den_in{bb}", name=f"di{bb}")
                  for bb in range(b)]
        den_dram = [nc.dram_tensor(f"den_dram{bb}", [n], BF16) for bb in range(b)]
        den_f32 = [const.tile([128, NJ], F32, tag=f"den_f32{bb}", name=f"df{bb}")
                   for bb in range(b)]
        recip_sb = [const.tile([128, NJ], F32, tag=f"recip{bb}", name=f"rc{bb}")
                    for bb in range(b)]

        # ---------------- qT tiles + projections (all batches) ----------------
        with tc.tile_pool(name="qt", bufs=2 * CC) as qt_pool, \
             tc.tile_pool(name="pqk", bufs=4, space="PSUM") as pqk_pool, \
             tc.tile_pool(name="pv", bufs=2, space="PSUM") as pv_pool:
            for bb in range(b):
                pair, lb = bb // 2, bb % 2
                rows = slice(64 * lb, 64 * lb + 64)
                qt_c = []
                for c in range(CC):
                    t = qt_pool.tile([128, n], BF16, tag="qt", name="qtc")
                    nc.sync.dma_start(out=t, in_=qT[c * 128:(c + 1) * 128,
                                                    bb * n:(bb + 1) * n])
                    qt_c.append(t)
                for ic in range(n // 512):
                    ps = pqk_pool.tile([128, 512], F32, tag="pqk")
                    for c in range(CC):
                        nc.tensor.matmul(
                            ps, lhsT=wqk_sb[:, c, :],
                            rhs=qt_c[c][:, ic * 512:(ic + 1) * 512],
                            start=(c == 0), stop=(c == CC - 1))
                    cols = slice(ic * 512, (ic + 1) * 512)
                    nc.vector.tensor_copy(qT_sb[pair][rows, cols], ps[0:64, :])
                    nc.vector.tensor_copy(kT_sb[pair][rows, cols], ps[64:128, :])
                for t4 in range(NJ // 4):
                    psv = pv_pool.tile([128, 4, DH], F32, tag="pv")
                    for k in range(4):
                        tt = t4 * 4 + k
                        for c in range(CC):
                            nc.tensor.matmul(
                                psv[:, k, :],
                                lhsT=qt_c[c][:, tt * 128:(tt + 1) * 128],
                                rhs=wv_sb[:, c, :],
                                start=(c == 0), stop=(c == CC - 1))
                    nc.vector.tensor_copy(
                        v_sb[bb][:, t4 * 4:(t4 + 1) * 4, 0:DH], psv)

        # ---------------- bias loads (all resident; 4 jt per tile) -------------
        bias_t = {}
        for ip in range(NIP):
            for jq in range(NJ // 4):
                t = bias_pool.tile([128, 4, 1024], BF16, tag=f"bias{ip}{jq}",
                                   name=f"bias{ip}{jq}")
                nc.sync.dma_start(
                    out=t,
                    in_=biasT[jq * 512:(jq + 1) * 512,
                              ip * 1024:(ip + 1) * 1024].rearrange(
                                  "(c p) i -> p c i", p=128))
                bias_t[(ip, jq)] = t

        # ---------------- scores + softmax + P~^T V + out-proj ----------------
        n_tiles = 0
        pe_quota = 0.0
        with tc.tile_pool(name="st", bufs=2, space="PSUM") as st_pool, \
             tc.tile_pool(name="ot", bufs=2, space="PSUM") as ot_pool, \
             tc.tile_pool(name="po", bufs=2, space="PSUM") as po_pool:
            for ip in range(NIP):
                for pair in range(NPAIR):
                    for lb in range(2):
                        bb = 2 * pair + lb
                        rows = slice(64 * lb, 64 * lb + 64)
                        ot_ps = [ot_pool.tile([VW, 512], F32, tag="ot", name="otp")
                                 for _ in range(2)]
                        for jt in range(NJ):
                            bt = bias_t[(ip, jt // 4)][:, jt % 4, :]
                            st = st_pool.tile([128, 1024], F32, tag="st")
                            # choose bias path to balance PE vs DVE
                            pe_quota += alpha
                            use_pe = pe_quota >= 1.0
                            if use_pe:
                                pe_quota -= 1.0
                            for il in range(2):
                                cols = slice(il * 512, (il + 1) * 512)
                                ic2 = ip * 2 + il
                                if use_pe:
                                    nc.tensor.matmul(
                                        st[:, cols], lhsT=ident, rhs=bt[:, cols],
                                        start=True, stop=False)
                                nc.tensor.matmul(
                                    st[:, cols],
                                    lhsT=kT_sb[pair][rows, jt * 128:(jt + 1) * 128],
                                    rhs=qT_sb[pair][rows, ic2 * 512:(ic2 + 1) * 512],
                                    start=not use_pe, stop=True)
                            if use_pe:
                                exp_in = st
                            else:
                                s_sb = s_pool.tile([128, 1024], F32, tag="s_sb")
                                nc.vector.scalar_tensor_tensor(
                                    s_sb, st, 0.0, bt,
                                    mybir.AluOpType.add, mybir.AluOpType.add)
                                exp_in = s_sb
                            pexp = p_pool.tile([128, 1024], BF16, tag="pexp")
                            nc.scalar.activation(
                                pexp, exp_in, mybir.ActivationFunctionType.Exp,
                                bias=zbias)
                            for il in range(2):
                                nc.tensor.matmul(
                                    ot_ps[il],
                                    lhsT=v_sb[bb][:, jt, :],
                                    rhs=pexp[:, il * 512:(il + 1) * 512],
                                    start=(jt == 0), stop=(jt == NJ - 1))
                            n_tiles += 1
                        # evacuate oT (+den row) for this (ip, pair, lb)
                        for il in range(2):
                            ccols = slice((ip * 2 + il) * 512, (ip * 2 + il + 1) * 512)
                            nc.vector.tensor_copy(ot65[pair][lb][:, ccols], ot_ps[il])
                        # ---- pipelined epilogue for this (ip, pair, lb) ----
                        # den row -> per-token-tile columns (via DRAM bounce)
                        nc.sync.dma_start(
                            out=den_dram[bb][ip * 1024:(ip + 1) * 1024],
                            in_=ot65[pair][lb][64:65, ip * 1024:(ip + 1) * 1024])
                        nc.sync.dma_start(
                            out=den_in[bb][:, ip * 8:(ip + 1) * 8],
                            in_=den_dram[bb][ip * 1024:(ip + 1) * 1024]
                            .rearrange("(t p) -> p t", p=128))
                        nc.vector.tensor_copy(
                            den_f32[bb][:, ip * 8:(ip + 1) * 8],
                            den_in[bb][:, ip * 8:(ip + 1) * 8])
                        nc.vector.reciprocal(
                            recip_sb[bb][:, ip * 8:(ip + 1) * 8],
                            den_f32[bb][:, ip * 8:(ip + 1) * 8])
                        for tg in range(8):
                            tgg = ip * 8 + tg
                            po = po_pool.tile([128, d], F32, tag="po")
                            nc.tensor.matmul(
                                po, lhsT=ot65[pair][lb][0:64, tgg * 128:(tgg + 1) * 128],
                                rhs=wout_sb, start=True, stop=True)
                            osb = out_pool.tile([128, d], BF16, tag="osb")
                            nc.vector.tensor_scalar_mul(
                                osb, po, recip_sb[bb][:, tgg:tgg + 1])
                            nc.sync.dma_start(
                                out=out[bb * n + tgg * 128: bb * n + (tgg + 1) * 128, :],
                                in_=osb)
    nc.compile()
    return nc


def make_in_maps(query, pos_bias, Wq, Wk, Wv, Wout, n_cores=N_CORES):
    """Host-side sharding/layout prep. Head h -> core h."""
    query = np.asarray(query, dtype=np.float32)
    pos_bias = np.asarray(pos_bias, dtype=np.float32)
    Wq = np.asarray(Wq, dtype=np.float32)
    Wk = np.asarray(Wk, dtype=np.float32)
    Wv = np.asarray(Wv, dtype=np.float32)
    Wout = np.asarray(Wout, dtype=np.float32)

    b, n, d = query.shape
    qT = np.ascontiguousarray(query.reshape(b * n, d).T).astype(BF16NP)
    wq_s = Wq * np.float32(SCALE)
    in_maps = []
    for h in range(n_cores):
        sl = slice(h * DH, (h + 1) * DH)
        wqk = np.concatenate([wq_s[:, sl], Wk[:, sl]], axis=1)
        in_maps.append({
            "qT": qT,
            "biasT": np.ascontiguousarray(pos_bias[h].T).astype(BF16NP),
            "wqk": np.ascontiguousarray(wqk).astype(BF16NP),
            "wv": np.ascontiguousarray(Wv[:, sl]).astype(BF16NP),
            "wout": np.ascontiguousarray(Wout[sl, :]).astype(BF16NP),
        })
    return in_maps


def run_device(in_maps, b=B, n=N, d=D, trace=False, **kw):
    nc = build_nc(b, n, d, n_cores=len(in_maps))
    return run_bass_kernel_spmd(nc, in_maps, list(range(len(in_maps))), trace=trace, **kw)


def assemble(results, b=B, n=N, d=D):
    acc = np.zeros((b * n, d), dtype=np.float32)
    for r in results:
        acc += np.asarray(r["out"], dtype=np.float32)
    return acc.reshape(b, n, d)


def kernel(query, pos_bias, Wq, Wk, Wv, Wout):
    in_maps = make_in_maps(query, pos_bias, Wq, Wk, Wv, Wout)
    res = run_device(in_maps)
    return assemble(res.results)


# revision 8
# speedup vs baseline: 1.2950x; 1.0852x over previous
"""Multi-head self-attention with positional bias, sharded over 8 NeuronCores.

Sharding: head-parallel. Core h computes head h for all batches; the full
output is the sum of the 8 per-core partials (row-parallel Wout), summed on
host in fp32.

v2 design (driven by the TimelineSim cost model, where a matmul costs
out_free_size * pe_cycle and engine element ops cost free_size * cycle_t):
  - everything bf16 on the wires (qT, bias, weights, pexp, oT, out); fp32
    only in PSUM accumulation and the exp input.
  - Wq/Wk merged into one [d, 128] projection matmul (halves proj MM count).
  - scores computed transposed ST[j, i] = k_j . q_i + bias[i, j]; the bias
    lands via EITHER an identity matmul on PE (start=True) OR a DVE
    scalar_tensor_tensor add staged through SBUF -- split by ALPHA to balance
    the PE and DVE engines.
  - exp on ACT (the hard floor: ~133us for 16.8M elements), 1024-wide ops.
  - softmax denominator: ones column 64 in v (costs nothing extra on PE);
    the oT evacuation keeps the den row in the same bf16 tile; a SBUF->SBUF
    transpose DMA turns den rows into per-token-tile columns for reciprocal.
  - loop order (ip, pair, lb) so only 2 oT accumulators are live -> PSUM fits
    st double-buffering (4) + ot (2) + out-proj po (2) = 8 banks.
  - out-projection + normalization + store pipelined per (ip, pair, lb).
"""

import numpy as np
import ml_dtypes
from contextlib import ExitStack

import concourse.bass as bass
import concourse.bacc as bacc
import concourse.mybir as mybir
import concourse.tile as tile
from concourse.bass_utils import run_bass_kernel_spmd
from concourse.masks import make_identity

HEADS = 8
DH = 64
B, N, D = 4, 2048, 512
SCALE = DH ** -0.5
N_CORES = 8

# fraction of (jt, lb) score tiles whose bias-add runs as a PE identity
# matmul; the rest run as DVE adds staged through SBUF.
ALPHA = 0.32

F32 = mybir.dt.float32
BF16 = mybir.dt.bfloat16
BF16NP = ml_dtypes.bfloat16


def build_nc(b=B, n=N, d=D, alpha=None, n_cores=1):
    """Build the per-core Bass program (SPMD; per-head data via inputs)."""
    if alpha is None:
        alpha = ALPHA
    assert b % 2 == 0 and n % 1024 == 0 and d % 128 == 0
    T = b * n
    CC = d // 128        # contraction chunks for projections
    NJ = n // 128        # key tiles (j)
    NIP = n // 1024      # i-windows of 1024
    NPAIR = b // 2
    VW = 65              # v block width (ones column at 64)

    nc = bacc.Bacc("TRN2", target_bir_lowering=False, debug=False,
                   num_devices=n_cores)
    qT = nc.declare_dram_parameter("qT", [d, T], BF16, isOutput=False)
    biasT = nc.declare_dram_parameter("biasT", [n, n], BF16, isOutput=False)
    wqk = nc.declare_dram_parameter("wqk", [d, 128], BF16, isOutput=False)
    wv = nc.declare_dram_parameter("wv", [d, DH], BF16, isOutput=False)
    wout = nc.declare_dram_parameter("wout", [DH, d], BF16, isOutput=False)
    out = nc.declare_dram_parameter("out", [T, d], BF16, isOutput=True)

    with ExitStack() as ctx:
        tc = ctx.enter_context(tile.TileContext(nc))

        const = ctx.enter_context(tc.tile_pool(name="const", bufs=1))
        qk_pool = ctx.enter_context(tc.tile_pool(name="qkT", bufs=1))
        v_pool = ctx.enter_context(tc.tile_pool(name="v", bufs=1))
        ot_sb_pool = ctx.enter_context(tc.tile_pool(name="ot_sb", bufs=1))
        bias_pool = ctx.enter_context(tc.tile_pool(name="bias", bufs=1))
        s_pool = ctx.enter_context(tc.tile_pool(name="s_sb", bufs=3))
        p_pool = ctx.enter_context(tc.tile_pool(name="pexp", bufs=4))
        out_pool = ctx.enter_context(tc.tile_pool(name="osb", bufs=6))

        ident_f32 = const.tile([128, 128], F32, tag="ident_f32")
        make_identity(nc, ident_f32)
        ident = const.tile([128, 128], BF16, tag="ident")
        nc.vector.tensor_copy(ident, ident_f32)
        zbias = const.tile([128, 1], F32, tag="zbias")
        nc.vector.memset(zbias, 0.0)

        wqk_sb = const.tile([128, CC, 128], BF16, tag="wqk")
        nc.sync.dma_start(out=wqk_sb, in_=wqk[:, :].rearrange("(c p) e -> p c e", p=128))
        wv_sb = const.tile([128, CC, DH], BF16, tag="wv")
        nc.sync.dma_start(out=wv_sb, in_=wv[:, :].rearrange("(c p) e -> p c e", p=128))
        wout_sb = const.tile([64, d], BF16, tag="wout")
        nc.sync.dma_start(out=wout_sb, in_=wout[:, :])

        qT_sb = [qk_pool.tile([128, n], BF16, tag=f"qT{p}", name=f"qT{p}") for p in range(NPAIR)]
        kT_sb = [qk_pool.tile([128, n], BF16, tag=f"kT{p}", name=f"kT{p}") for p in range(NPAIR)]
        v_sb = [v_pool.tile([128, NJ, VW], BF16, tag=f"v{bb}", name=f"v{bb}") for bb in range(b)]
        for bb in range(b):
            nc.vector.memset(v_sb[bb][:, :, DH:VW], 1.0)
        # oT + den row, per (pair, lb): rows 0..63 = oT (dh), row 64 = denom
        ot65 = [[ot_sb_pool.tile([VW, n], BF16, tag=f"ot{p}{l}", name=f"ot{p}{l}")
                 for l in range(2)] for p in range(NPAIR)]
        den_in = [const.tile([128, NJ], BF16, tag=f"den_in{bb}", name=f"di{bb}")
                  for bb in range(b)]
        den_dram = [nc.dram_tensor(f"den_dram{bb}", [n], BF16) for bb in range(b)]
        den_f32 = [const.tile([128, NJ], F32, tag=f"den_f32{bb}", name=f"df{bb}")
                   for bb in range(b)]
        recip_sb = [const.tile([128, NJ], F32, tag=f"recip{bb}", name=f"rc{bb}")
                    for bb in range(b)]

        # ---------------- qT tiles + projections (all batches) ----------------
        # DMA order: qt(bb0), qt(bb1), bias(ip0), qt(bb2), qt(bb3), bias(ip1)
        # so the first score block can start as soon as bb0/bb1 are projected.
        bias_t = {}

        def load_bias(ip):
            for jq in range(NJ // 4):
                t = bias_pool.tile([128, 4, 1024], BF16, tag=f"bias{ip}{jq}",
                                   name=f"bias{ip}{jq}")
                nc.sync.dma_start(
                    out=t,
                    in_=biasT[jq * 512:(jq + 1) * 512,
                              ip * 1024:(ip + 1) * 1024].rearrange(
                                  "(c p) i -> p c i", p=128))
                bias_t[(ip, jq)] = t

        with tc.tile_pool(name="qt", bufs=2 * CC) as qt_pool, \
             tc.tile_pool(name="pqk", bufs=4, space="PSUM") as pqk_pool, \
             tc.tile_pool(name="pv", bufs=2, space="PSUM") as pv_pool:
            for bb in range(b):
                pair, lb = bb // 2, bb % 2
                rows = slice(64 * lb, 64 * lb + 64)
                qt_c = []
                for c in range(CC):
                    t = qt_pool.tile([128, n], BF16, tag="qt", name="qtc")
                    nc.sync.dma_start(out=t, in_=qT[c * 128:(c + 1) * 128,
                                                    bb * n:(bb + 1) * n])
                    qt_c.append(t)
                if bb == 2:
                    load_bias(0)
                for ic in range(n // 512):
                    ps = pqk_pool.tile([128, 512], F32, tag="pqk")
                    for c in range(CC):
                        nc.tensor.matmul(
                            ps, lhsT=wqk_sb[:, c, :],
                            rhs=qt_c[c][:, ic * 512:(ic + 1) * 512],
                            start=(c == 0), stop=(c == CC - 1))
                    cols = slice(ic * 512, (ic + 1) * 512)
                    nc.vector.tensor_copy(qT_sb[pair][rows, cols], ps[0:64, :])
                    nc.vector.tensor_copy(kT_sb[pair][rows, cols], ps[64:128, :])
                for t4 in range(NJ // 4):
                    psv = pv_pool.tile([128, 4, DH], F32, tag="pv")
                    for k in range(4):
                        tt = t4 * 4 + k
                        for c in range(CC):
                            nc.tensor.matmul(
                                psv[:, k, :],
                                lhsT=qt_c[c][:, tt * 128:(tt + 1) * 128],
                                rhs=wv_sb[:, c, :],
                                start=(c == 0), stop=(c == CC - 1))
                    nc.vector.tensor_copy(
                        v_sb[bb][:, t4 * 4:(t4 + 1) * 4, 0:DH], psv)
            load_bias(1)

        # ---------------- scores + softmax + P~^T V + out-proj ----------------
        pe_quota = [0.0]
        act_quota = [0.0]
        GAMMA = 0.45  # fraction of output normalizations on ACT (rest DVE)

        with tc.tile_pool(name="st", bufs=2, space="PSUM") as st_pool, \
             tc.tile_pool(name="ot", bufs=2, space="PSUM") as ot_pool, \
             tc.tile_pool(name="po", bufs=2, space="PSUM") as po_pool:

            def out_proj(ip, pair, lb):
                """Out-projection + normalize + store for one (ip, pair, lb)."""
                bb = 2 * pair + lb
                for tg in range(8):
                    tgg = ip * 8 + tg
                    po = po_pool.tile([128, d], F32, tag="po")
                    nc.tensor.matmul(
                        po, lhsT=ot65[pair][lb][0:64, tgg * 128:(tgg + 1) * 128],
                        rhs=wout_sb, start=True, stop=True)
                    osb = out_pool.tile([128, d], BF16, tag="osb")
                    act_quota[0] += GAMMA
                    if act_quota[0] >= 1.0:
                        act_quota[0] -= 1.0
                        nc.scalar.mul(osb, po, recip_sb[bb][:, tgg:tgg + 1])
                    else:
                        nc.vector.tensor_scalar_mul(
                            osb, po, recip_sb[bb][:, tgg:tgg + 1])
                    nc.sync.dma_start(
                        out=out[bb * n + tgg * 128: bb * n + (tgg + 1) * 128, :],
                        in_=osb)

            pending = None
            for ip in range(NIP):
                for pair in range(NPAIR):
                    for lb in range(2):
                        bb = 2 * pair + lb
                        rows = slice(64 * lb, 64 * lb + 64)
                        ot_ps = [ot_pool.tile([VW, 512], F32, tag="ot", name="otp")
                                 for _ in range(2)]
                        for jt in range(NJ):
                            if jt == 4 and pending is not None:
                                out_proj(*pending)
                                pending = None
                            bt = bias_t[(ip, jt // 4)][:, jt % 4, :]
                            st = st_pool.tile([128, 1024], F32, tag="st")
                            # choose bias path to balance PE vs DVE
                            pe_quota[0] += alpha
                            use_pe = pe_quota[0] >= 1.0
                            if use_pe:
                                pe_quota[0] -= 1.0
                            for il in range(2):
                                cols = slice(il * 512, (il + 1) * 512)
                                ic2 = ip * 2 + il
                                if use_pe:
                                    nc.tensor.matmul(
                                        st[:, cols], lhsT=ident, rhs=bt[:, cols],
                                        start=True, stop=False)
                                nc.tensor.matmul(
                                    st[:, cols],
                                    lhsT=kT_sb[pair][rows, jt * 128:(jt + 1) * 128],
                                    rhs=qT_sb[pair][rows, ic2 * 512:(ic2 + 1) * 512],
                                    start=not use_pe, stop=True)
                            if use_pe:
                                exp_in = st
                            else:
                                s_sb = s_pool.tile([128, 1024], F32, tag="s_sb")
                                nc.vector.scalar_tensor_tensor(
                                    s_sb, st, 0.0, bt,
                                    mybir.AluOpType.add, mybir.AluOpType.add)
                                exp_in = s_sb
                            pexp = p_pool.tile([128, 1024], BF16, tag="pexp")
                            nc.scalar.activation(
                                pexp, exp_in, mybir.ActivationFunctionType.Exp,
                                bias=zbias)
                            for il in range(2):
                                nc.tensor.matmul(
                                    ot_ps[il],
                                    lhsT=v_sb[bb][:, jt, :],
                                    rhs=pexp[:, il * 512:(il + 1) * 512],
                                    start=(jt == 0), stop=(jt == NJ - 1))
                        # evacuate oT (+den row) for this (ip, pair, lb)
                        for il in range(2):
                            ccols = slice((ip * 2 + il) * 512, (ip * 2 + il + 1) * 512)
                            nc.vector.tensor_copy(ot65[pair][lb][:, ccols], ot_ps[il])
                        # den row -> per-token-tile columns (via DRAM bounce);
                        # start the round-trip now, emit the out-proj later so
                        # its latency hides under the next block's score work.
                        nc.sync.dma_start(
                            out=den_dram[bb][ip * 1024:(ip + 1) * 1024],
                            in_=ot65[pair][lb][64:65, ip * 1024:(ip + 1) * 1024])
                        nc.sync.dma_start(
                            out=den_in[bb][:, ip * 8:(ip + 1) * 8],
                            in_=den_dram[bb][ip * 1024:(ip + 1) * 1024]
                            .rearrange("(t p) -> p t", p=128))
                        nc.vector.tensor_copy(
                            den_f32[bb][:, ip * 8:(ip + 1) * 8],
                            den_in[bb][:, ip * 8:(ip + 1) * 8])
                        nc.vector.reciprocal(
                            recip_sb[bb][:, ip * 8:(ip + 1) * 8],
                            den_f32[bb][:, ip * 8:(ip + 1) * 8])
                        pending = (ip, pair, lb)
            out_proj(*pending)
    nc.compile()
    return nc


def make_in_maps(query, pos_bias, Wq, Wk, Wv, Wout, n_cores=N_CORES):
    """Host-side sharding/layout prep. Head h -> core h."""
    query = np.asarray(query, dtype=np.float32)
    pos_bias = np.asarray(pos_bias, dtype=np.float32)
    Wq = np.asarray(Wq, dtype=np.float32)
    Wk = np.asarray(Wk, dtype=np.float32)
    Wv = np.asarray(Wv, dtype=np.float32)
    Wout = np.asarray(Wout, dtype=np.float32)

    b, n, d = query.shape
    qT = np.ascontiguousarray(query.reshape(b * n, d).T).astype(BF16NP)
    wq_s = Wq * np.float32(SCALE)
    in_maps = []
    for h in range(n_cores):
        sl = slice(h * DH, (h + 1) * DH)
        wqk = np.concatenate([wq_s[:, sl], Wk[:, sl]], axis=1)
        in_maps.append({
            "qT": qT,
            "biasT": np.ascontiguousarray(pos_bias[h].T).astype(BF16NP),
            "wqk": np.ascontiguousarray(wqk).astype(BF16NP),
            "wv": np.ascontiguousarray(Wv[:, sl]).astype(BF16NP),
            "wout": np.ascontiguousarray(Wout[sl, :]).astype(BF16NP),
        })
    return in_maps


def run_device(in_maps, b=B, n=N, d=D, trace=False, **kw):
    nc = build_nc(b, n, d, n_cores=len(in_maps))
    return run_bass_kernel_spmd(nc, in_maps, list(range(len(in_maps))), trace=trace, **kw)


def assemble(results, b=B, n=N, d=D):
    acc = np.zeros((b * n, d), dtype=np.float32)
    for r in results:
        acc += np.asarray(r["out"], dtype=np.float32)
    return acc.reshape(b, n, d)


def kernel(query, pos_bias, Wq, Wk, Wv, Wout):
    in_maps = make_in_maps(query, pos_bias, Wq, Wk, Wv, Wout)
    res = run_device(in_maps)
    return assemble(res.results)
